# revision 19
# baseline (speedup 1.0000x reference)
"""Trainium2 Bass kernel for CausalGNNRecommender.

Full inputs in, full outputs out. Internally: shard the N=16384 node dim
across 8 NeuronCores (2048 rows each) and run a fused
attention + 2x SAGE-conv kernel per core, with AllGather collectives
between layers.

Design notes:
- Attention is flash-style in column-major layout (H=64 on partitions):
  scoresT chunks [128 keys, 512 rows] on the PE (two K=64 matmuls packed
  into disjoint PE row-groups), exp on ACT batched [128, 1024], then
  PSUM-accumulated attn @ x with a ones-column giving softmax
  normalizers for free (no max subtraction needed: |scores| <= ~35 so
  exp stays in fp32 range).
- SAGE segment-sum is a dense SpMM: the host re-encodes edge_index as a
  per-core bf16 count matrix A [N, 2048] (counts are exact in bf16);
  the device streams A as the matmul moving operand against the node
  features split as x = hi + lo bf16 stationaries (recovers full fp32
  precision). A ones-column in x_hi yields in-degrees for free.
"""

import os
import sys

sys.path.insert(0, "/opt/trn_rl_repo")

import numpy as np
import ml_dtypes

import concourse.bacc as bacc
import concourse.tile as tile
from concourse import mybir
from concourse.bass_utils import run_bass_kernel_spmd
from concourse.masks import make_identity

# Problem shapes (hardcoded per harness contract).
U, I, H, E = 4096, 12288, 64, 524288
N = U + I              # 16384
CORES = 8
NL = N // CORES        # 2048 rows per core
NB = NL // 128         # 16 node blocks per core
KC = N // 128          # 128 key chunks
G = NL // 512          # 4 groups of 512 rows

f32 = mybir.dt.float32
bf16 = mybir.dt.bfloat16

# Results of the last device run (exec time etc.) for external inspection.
LAST_RESULTS = None
_NC_CACHE = {}

# wpack columns
WC_ATTN = slice(0, 64)
WC_LW = [slice(64, 128), slice(192, 256)]
WC_RW = [slice(128, 192), slice(256, 320)]
WC_AB = slice(320, 321)
WC_LB = [slice(321, 322), slice(322, 323)]


def _build_nc():
    """Build the single-core SPMD Bass program (identical on all 8 cores)."""
    nc = bacc.Bacc("TRN2", target_bir_lowering=False, debug=False)

    t_x0aug = nc.dram_tensor("x0aug", [N, H + 1], f32, kind="ExternalInput")
    t_xT0 = nc.dram_tensor("xT0", [H, NL], f32, kind="ExternalInput")
    t_adj2 = nc.dram_tensor("adj2", [128, N // 2], f32, kind="ExternalInput")
    t_wpack = nc.dram_tensor("wpack", [H, 323], f32, kind="ExternalInput")
    t_A = nc.dram_tensor("A", [N, NL], bf16, kind="ExternalInput")
    t_out = nc.dram_tensor("out", [NL, H], f32, kind="ExternalOutput")

    d_sh = [nc.dram_tensor(f"sh{l}", [NL, H], f32) for l in range(2)]
    d_tb = [
        nc.dram_tensor(f"tb{l}", [N, H], f32, addr_space="Shared")
        for l in range(2)
    ]

    with tile.TileContext(nc) as tc:
        with tc.tile_pool(name="consts", bufs=1) as consts:
            ident = consts.tile([128, 128], f32)
            make_identity(nc, ident[:])
            wpack = consts.tile([H, 323], f32)
            nc.sync.dma_start(wpack[:], t_wpack[:])
            xT0 = consts.tile([H, NL], f32)
            nc.sync.dma_start(xT0[:], t_xT0[:])
            x_a = consts.tile([H, NL], f32)  # x1T (attention out, col layout)
            x_b = consts.tile([H, NL], f32)  # x2T (layer-0 out, col layout)

            # ---------------- attention ----------------
            with (
                tc.tile_pool(name="attn_sb", bufs=1) as asb,
                tc.tile_pool(name="exp_sb", bufs=6) as esb,
                tc.tile_pool(name="ps_out", bufs=1, space="PSUM") as pso,
            ):
                x0aug = asb.tile([128, KC, H + 1], f32)
                nc.sync.dma_start(
                    x0aug[:], t_x0aug.ap().rearrange("(k p) h -> p k h", p=128)
                )
                adj2 = asb.tile([128, N // 2], f32)
                nc.sync.dma_start(adj2[:], t_adj2[:])

                # xWT = attn_w @ xT0 + b, duplicated onto partitions 64:128
                xWT = asb.tile([128, NL], f32)
                with tc.tile_pool(name="ps_w", bufs=2, space="PSUM") as psw:
                    for g in range(G):
                        gs = slice(g * 512, (g + 1) * 512)
                        ps = psw.tile([128, 512], f32, tag="xw")
                        nc.tensor.matmul(ps[0:64, :], wpack[:, WC_ATTN], xT0[:, gs])
                        nc.tensor.matmul(ps[64:128, :], wpack[:, WC_ATTN], xT0[:, gs])
                        nc.scalar.activation(
                            xWT[0:64, gs],
                            ps[0:64, :],
                            mybir.ActivationFunctionType.Identity,
                            bias=wpack[:, WC_AB],
                        )
                        nc.scalar.activation(
                            xWT[64:128, gs],
                            ps[64:128, :],
                            mybir.ActivationFunctionType.Identity,
                            bias=wpack[:, WC_AB],
                        )

                out_ps = [
                    pso.tile([H + 1, 512], f32, tag=f"o{g}", name=f"out_ps{g}")
                    for g in range(G)
                ]

                # main loop: k2 indexes pairs of key chunks (2*k2, 2*k2+1)
                with tc.tile_pool(name="ps_sc", bufs=2, space="PSUM") as pss:
                    for k2 in range(KC // 2):
                        ks = slice(k2 * 128, (k2 + 1) * 128)
                        exs = []
                        for g in range(G):
                            gs = slice(g * 512, (g + 1) * 512)
                            sc = pss.tile([128, 2, 512], f32, tag="sc")
                            # two K=64 matmuls packed into PE row groups 0/64
                            nc.tensor.matmul(
                                sc[:, 0, :], adj2[0:64, ks], xWT[0:64, gs]
                            )
                            nc.tensor.matmul(
                                sc[:, 1, :], adj2[64:128, ks], xWT[64:128, gs]
                            )
                            ex = esb.tile([128, 2, 512], f32, tag="ex", bufs=6)
                            nc.scalar.activation(
                                ex[:], sc[:], mybir.ActivationFunctionType.Exp
                            )
                            exs.append(ex)
                        for g in range(G):
                            nc.tensor.matmul(
                                out_ps[g][:],
                                x0aug[:, 2 * k2, :],
                                exs[g][:, 0, :],
                                start=(k2 == 0),
                                stop=False,
                            )
                            nc.tensor.matmul(
                                out_ps[g][:],
                                x0aug[:, 2 * k2 + 1, :],
                                exs[g][:, 1, :],
                                start=False,
                                stop=(k2 == KC // 2 - 1),
                            )

                # softmax divide + build x1 rows and x1T
                x1rows = asb.tile([128, NB, H], f32)
                with tc.tile_pool(name="ps_tr", bufs=2, space="PSUM") as pst:
                    for g in range(G):
                        ot = esb.tile([H + 1, 512], f32, tag="ot", bufs=2)
                        nc.scalar.copy(ot[:], out_ps[g][:])
                        for i in range(4):
                            j = g * 4 + i
                            pr = pst.tile([128, H + 1], f32, tag="pr")
                            nc.tensor.transpose(
                                pr[:],
                                ot[:, i * 128 : (i + 1) * 128],
                                ident[0 : H + 1, 0 : H + 1],
                            )
                            r = esb.tile([128, 1], f32, tag="r", bufs=2)
                            nc.vector.reciprocal(r[:], pr[:, H : H + 1])
                            nc.vector.tensor_scalar_mul(
                                x1rows[:, j, :], pr[:, 0:H], r[:]
                            )
                            pt = pst.tile([H, 128], f32, tag="pt")
                            nc.tensor.transpose(pt[:], x1rows[:, j, :], ident[:])
                            nc.vector.tensor_copy(
                                x_a[:, j * 128 : (j + 1) * 128], pt[:]
                            )
                nc.sync.dma_start(
                    d_sh[0].ap().rearrange("(c p) h -> p c h", p=128), x1rows[:]
                )

            # ---------------- SAGE layers (dense-A SpMM) ----------------
            for layer in range(2):
                x_in = x_a if layer == 0 else x_b
                x_out = x_b if layer == 0 else x_a

                nc.gpsimd.collective_compute(
                    "AllGather",
                    mybir.AluOpType.bypass,
                    replica_groups=[list(range(CORES))],
                    ins=[d_sh[layer][:]],
                    outs=[d_tb[layer][:]],
                )

                with (
                    tc.tile_pool(name=f"sage_sb{layer}", bufs=1) as ssb,
                    tc.tile_pool(name=f"sageA{layer}", bufs=3) as sA,
                    tc.tile_pool(name=f"ps_ag{layer}", bufs=1, space="PSUM") as pag,
                    tc.tile_pool(name=f"ps_t{layer}", bufs=1, space="PSUM") as pt2,
                    tc.tile_pool(name=f"ps_l{layer}", bufs=2, space="PSUM") as pl2,
                ):
                    # load the all-gathered table; build bf16 hi/lo split with
                    # ones (hi) / zeros (lo) in the augmented column
                    xtbl = ssb.tile([128, KC, H], f32)
                    nc.sync.dma_start(
                        xtbl[:], d_tb[layer].ap().rearrange("(k p) h -> p k h", p=128)
                    )
                    xhi = ssb.tile([128, KC, H + 1], bf16)
                    xlo = ssb.tile([128, KC, H + 1], bf16)
                    nc.vector.tensor_copy(xhi[:, :, 0:H], xtbl[:])
                    nc.vector.tensor_tensor(
                        xlo[:, :, 0:H],
                        xtbl[:],
                        xhi[:, :, 0:H],
                        op=mybir.AluOpType.subtract,
                    )
                    nc.vector.memset(xhi[:, :, H : H + 1], 1.0)
                    nc.vector.memset(xlo[:, :, H : H + 1], 0.0)

                    ag_ps = [
                        pag.tile([H + 1, 512], f32, tag=f"a{g}", name=f"ag_ps{layer}{g}")
                        for g in range(G)
                    ]
                    for k in range(KC):
                        Ak = sA.tile([128, NL], bf16, tag="Ak", name=f"Ak{layer}_{k}")
                        nc.sync.dma_start(Ak[:], t_A[k * 128 : (k + 1) * 128, :])
                        for g in range(G):
                            gs = slice(g * 512, (g + 1) * 512)
                            nc.tensor.matmul(
                                ag_ps[g][:],
                                xhi[:, k, :],
                                Ak[:, gs],
                                start=(k == 0),
                                stop=False,
                            )
                        for g in range(G):
                            gs = slice(g * 512, (g + 1) * 512)
                            nc.tensor.matmul(
                                ag_ps[g][:],
                                xlo[:, k, :],
                                Ak[:, gs],
                                start=False,
                                stop=(k == KC - 1),
                            )

                    # divide by deg (row H of each group), rebuild col layout
                    aggrT = ssb.tile([H, NL], f32)
                    for g in range(G):
                        ot2 = ssb.tile([H + 1, 512], f32, tag="ot2", bufs=2)
                        nc.scalar.copy(ot2[:], ag_ps[g][:])
                        for i in range(4):
                            j = g * 4 + i
                            pr2 = pt2.tile([128, H + 1], f32, tag="tp", bufs=2)
                            nc.tensor.transpose(
                                pr2[:],
                                ot2[:, i * 128 : (i + 1) * 128],
                                ident[0 : H + 1, 0 : H + 1],
                            )
                            dmax = ssb.tile([128, 1], f32, tag="dmax", bufs=2)
                            nc.vector.tensor_scalar_max(
                                dmax[:], pr2[:, H : H + 1], 1.0
                            )
                            drec = ssb.tile([128, 1], f32, tag="drec", bufs=2)
                            nc.vector.reciprocal(drec[:], dmax[:])
                            arows = ssb.tile([128, H], f32, tag="arows", bufs=2)
                            nc.vector.tensor_scalar_mul(
                                arows[:], pr2[:, 0:H], drec[:]
                            )
                            pt3 = pt2.tile([H, 128], f32, tag="tp", bufs=2, name=f"pt3_{layer}_{j}")
                            nc.tensor.transpose(pt3[:], arows[:], ident[:])
                            nc.vector.tensor_copy(
                                aggrT[:, j * 128 : (j + 1) * 128], pt3[:]
                            )

                    # sage linear + relu (col layout)
                    for g in range(G):
                        gs = slice(g * 512, (g + 1) * 512)
                        ps2 = pl2.tile([H, 512], f32, tag="sage")
                        nc.tensor.matmul(
                            ps2[:], wpack[:, WC_LW[layer]], aggrT[:, gs],
                            start=True, stop=False,
                        )
                        nc.tensor.matmul(
                            ps2[:], wpack[:, WC_RW[layer]], x_in[:, gs],
                            start=False, stop=True,
                        )
                        nc.scalar.activation(
                            x_out[:, gs],
                            ps2[:],
                            mybir.ActivationFunctionType.Relu,
                            bias=wpack[:, WC_LB[layer]],
                        )

                    # rows + store
                    xrows = ssb.tile([128, NB, H], f32, tag=f"xrows{layer}")
                    for j in range(NB):
                        pr3 = pt2.tile([128, H], f32, tag="tp", bufs=2, name=f"pr3_{layer}_{j}")
                        nc.tensor.transpose(
                            pr3[:],
                            x_out[:, j * 128 : (j + 1) * 128],
                            ident[0:H, 0:H],
                        )
                        nc.vector.tensor_copy(xrows[:, j, :], pr3[:])
                    dst = d_sh[1] if layer == 0 else t_out
                    nc.sync.dma_start(
                        dst.ap().rearrange("(c p) h -> p c h", p=128), xrows[:]
                    )

    nc.finalize()
    return nc


def _build_A(edge_index):
    """Per-core dense bf16 count matrices A[c][src, tgt_local]."""
    src = np.asarray(edge_index[0], dtype=np.int64)
    tgt = np.asarray(edge_index[1], dtype=np.int64)
    c = tgt // NL
    tloc = tgt % NL
    flat = (c * N + src) * NL + tloc
    uf, cnt = np.unique(flat, return_counts=True)
    A8 = np.zeros(CORES * N * NL, dtype=np.uint16)
    A8[uf] = cnt.astype(np.uint16)
    return A8.reshape(CORES, N, NL).astype(ml_dtypes.bfloat16)


def kernel(edge_index, user_emb, item_emb, attn_w, attn_b, causal_adj,
           l0_lw, l0_lb, l0_rw, l1_lw, l1_lb, l1_rw):
    global LAST_RESULTS
    edge_index = np.asarray(edge_index)
    user_emb = np.asarray(user_emb, dtype=np.float32)
    item_emb = np.asarray(item_emb, dtype=np.float32)
    attn_w = np.asarray(attn_w, dtype=np.float32)
    attn_b = np.asarray(attn_b, dtype=np.float32)
    causal_adj = np.asarray(causal_adj, dtype=np.float32)

    A = _build_A(edge_index)

    x0 = np.concatenate([user_emb, item_emb], axis=0)  # [N, H]
    x0aug = np.ascontiguousarray(
        np.concatenate([x0, np.ones((N, 1), np.float32)], axis=1)
    )
    # adj packed for PE row-group pairing: even key chunks on partitions
    # 0:64, odd chunks on 64:128
    a4 = causal_adj.reshape(H, KC // 2, 2, 128)
    adj2 = np.ascontiguousarray(
        np.concatenate([a4[:, :, 0, :], a4[:, :, 1, :]], axis=0).reshape(128, N // 2)
    )

    wpack = np.zeros((H, 323), dtype=np.float32)
    wpack[:, 0:64] = attn_w.T
    wpack[:, 64:128] = np.asarray(l0_lw, np.float32).T
    wpack[:, 128:192] = np.asarray(l0_rw, np.float32).T
    wpack[:, 192:256] = np.asarray(l1_lw, np.float32).T
    wpack[:, 256:320] = np.asarray(l1_rw, np.float32).T
    wpack[:, 320] = attn_b
    wpack[:, 321] = np.asarray(l0_lb, np.float32)
    wpack[:, 322] = np.asarray(l1_lb, np.float32)

    if "nc" not in _NC_CACHE:
        _NC_CACHE["nc"] = _build_nc()
    nc = _NC_CACHE["nc"]

    in_maps = []
    for c in range(CORES):
        xT0c = np.ascontiguousarray(x0[c * NL : (c + 1) * NL].T)
        in_maps.append(
            {
                "x0aug": x0aug,
                "xT0": xT0c,
                "adj2": adj2,
                "wpack": wpack,
                "A": A[c],
            }
        )

    res = run_bass_kernel_spmd(
        nc,
        in_maps,
        core_ids=list(range(CORES)),
        trace=bool(os.environ.get("KERNEL_TRACE")),
    )
    LAST_RESULTS = res

    out = np.concatenate([res.results[c]["out"] for c in range(CORES)], axis=0)
    return out[:U], out[U:]


# revision 21
# speedup vs baseline: 1.4666x; 1.4666x over previous
"""Trainium2 Bass kernel for CausalGNNRecommender.

Full inputs in, full outputs out. Internally: shard the N=16384 node dim
across 8 NeuronCores (2048 rows each) and run a fused
attention + 2x SAGE-conv kernel per core, with AllGather collectives
between layers.

Design notes:
- Attention is flash-style in column-major layout (H=64 on partitions):
  scoresT chunks [128 keys, 512 rows] on the PE (two K=64 matmuls packed
  into disjoint PE row-groups), exp on ACT batched [128, 1024], then
  PSUM-accumulated attn @ x with a ones-column giving softmax
  normalizers for free (no max subtraction needed: |scores| <= ~35 so
  exp stays in fp32 range).
- SAGE segment-sum is a dense SpMM: the host re-encodes edge_index as a
  per-core bf16 count matrix A [N, 2048] (counts are exact in bf16);
  the device streams A as the matmul moving operand against the node
  features split as x = hi + lo bf16 stationaries (recovers full fp32
  precision). A ones-column in x_hi yields in-degrees for free.
"""

import os
import sys

sys.path.insert(0, "/opt/trn_rl_repo")

import numpy as np
import ml_dtypes

import concourse.bacc as bacc
import concourse.tile as tile
from concourse import mybir
from concourse.bass_utils import run_bass_kernel_spmd
from concourse.masks import make_identity

# Problem shapes (hardcoded per harness contract).
U, I, H, E = 4096, 12288, 64, 524288
N = U + I              # 16384
CORES = 8
NL = N // CORES        # 2048 rows per core
NB = NL // 128         # 16 node blocks per core
KC = N // 128          # 128 key chunks
G = NL // 512          # 4 groups of 512 rows

f32 = mybir.dt.float32
bf16 = mybir.dt.bfloat16

# Results of the last device run (exec time etc.) for external inspection.
LAST_RESULTS = None
_NC_CACHE = {}

# wpack columns
WC_ATTN = slice(0, 64)
WC_LW = [slice(64, 128), slice(192, 256)]
WC_RW = [slice(128, 192), slice(256, 320)]
WC_AB = slice(320, 321)
WC_LB = [slice(321, 322), slice(322, 323)]


def _build_nc():
    """Build the single-core SPMD Bass program (identical on all 8 cores)."""
    nc = bacc.Bacc("TRN2", target_bir_lowering=False, debug=False)

    t_x0aug = nc.dram_tensor("x0aug", [N, H + 1], f32, kind="ExternalInput")
    t_xT0 = nc.dram_tensor("xT0", [H, NL], f32, kind="ExternalInput")
    t_adj2 = nc.dram_tensor("adj2", [128, N // 2], f32, kind="ExternalInput")
    t_wpack = nc.dram_tensor("wpack", [H, 323], f32, kind="ExternalInput")
    t_A = nc.dram_tensor("A", [N, NL], bf16, kind="ExternalInput")
    t_out = nc.dram_tensor("out", [NL, H], f32, kind="ExternalOutput")

    d_sh = [nc.dram_tensor(f"sh{l}", [NL, H], f32) for l in range(2)]
    d_tb = [
        nc.dram_tensor(f"tb{l}", [N, H], f32, addr_space="Shared")
        for l in range(2)
    ]

    with tile.TileContext(nc) as tc:
        with tc.tile_pool(name="consts", bufs=1) as consts:
            ident = consts.tile([128, 128], f32)
            make_identity(nc, ident[:])
            wpack = consts.tile([H, 323], f32)
            nc.sync.dma_start(wpack[:], t_wpack[:])
            xT0 = consts.tile([H, NL], f32)
            nc.sync.dma_start(xT0[:], t_xT0[:])
            x_a = consts.tile([H, NL], f32)  # x1T (attention out, col layout)
            x_b = consts.tile([H, NL], f32)  # x2T (layer-0 out, col layout)

            # ---------------- attention ----------------
            with (
                tc.tile_pool(name="attn_sb", bufs=1) as asb,
                tc.tile_pool(name="exp_sb", bufs=6) as esb,
                tc.tile_pool(name="ps_out", bufs=1, space="PSUM") as pso,
            ):
                f32r = mybir.dt.float32r
                x0aug = asb.tile([128, KC, H + 1], f32r)
                adj2 = asb.tile([128, N // 2], f32r)
                with tc.tile_pool(name="raw_sb", bufs=1) as rsb:
                    x0aug_raw = rsb.tile([128, KC, H + 1], f32, tag="rx")
                    nc.sync.dma_start(
                        x0aug_raw[:],
                        t_x0aug.ap().rearrange("(k p) h -> p k h", p=128),
                    )
                    nc.vector.tensor_copy(x0aug[:], x0aug_raw[:])
                    adj2_raw = rsb.tile([128, N // 2], f32, tag="ra")
                    nc.sync.dma_start(adj2_raw[:], t_adj2[:])
                    nc.vector.tensor_copy(adj2[:], adj2_raw[:])

                # xWT = attn_w @ xT0 + b, duplicated onto partitions 64:128
                xWT = asb.tile([128, NL], f32r)
                with tc.tile_pool(name="ps_w", bufs=2, space="PSUM") as psw:
                    for g in range(G):
                        gs = slice(g * 512, (g + 1) * 512)
                        ps = psw.tile([128, 512], f32, tag="xw")
                        nc.tensor.matmul(ps[0:64, :], wpack[:, WC_ATTN], xT0[:, gs])
                        nc.tensor.matmul(ps[64:128, :], wpack[:, WC_ATTN], xT0[:, gs])
                        nc.scalar.activation(
                            xWT[0:64, gs],
                            ps[0:64, :],
                            mybir.ActivationFunctionType.Identity,
                            bias=wpack[:, WC_AB],
                        )
                        nc.scalar.activation(
                            xWT[64:128, gs],
                            ps[64:128, :],
                            mybir.ActivationFunctionType.Identity,
                            bias=wpack[:, WC_AB],
                        )

                out_ps = [
                    pso.tile([H + 1, 512], f32, tag=f"o{g}", name=f"out_ps{g}")
                    for g in range(G)
                ]

                # main loop: k2 indexes pairs of key chunks (2*k2, 2*k2+1)
                with tc.tile_pool(name="ps_sc", bufs=2, space="PSUM") as pss:
                    for k2 in range(KC // 2):
                        ks = slice(k2 * 128, (k2 + 1) * 128)
                        exs = []
                        for g in range(G):
                            gs = slice(g * 512, (g + 1) * 512)
                            sc = pss.tile([128, 2, 512], f32, tag="sc")
                            # two K=64 matmuls packed into PE row groups 0/64
                            nc.tensor.matmul(
                                sc[:, 0, :], adj2[0:64, ks], xWT[0:64, gs]
                            )
                            nc.tensor.matmul(
                                sc[:, 1, :], adj2[64:128, ks], xWT[64:128, gs]
                            )
                            ex = esb.tile([128, 2, 512], f32r, tag="ex", bufs=6)
                            nc.scalar.activation(
                                ex[:], sc[:], mybir.ActivationFunctionType.Exp
                            )
                            exs.append(ex)
                        for g in range(G):
                            nc.tensor.matmul(
                                out_ps[g][:],
                                x0aug[:, 2 * k2, :],
                                exs[g][:, 0, :],
                                start=(k2 == 0),
                                stop=False,
                            )
                            nc.tensor.matmul(
                                out_ps[g][:],
                                x0aug[:, 2 * k2 + 1, :],
                                exs[g][:, 1, :],
                                start=False,
                                stop=(k2 == KC // 2 - 1),
                            )

                # softmax divide + build x1 rows and x1T
                x1rows = asb.tile([128, NB, H], f32)
                with tc.tile_pool(name="ps_tr", bufs=2, space="PSUM") as pst:
                    for g in range(G):
                        ot = esb.tile([H + 1, 512], f32, tag="ot", bufs=2)
                        nc.scalar.copy(ot[:], out_ps[g][:])
                        for i in range(4):
                            j = g * 4 + i
                            pr = pst.tile([128, H + 1], f32, tag="pr")
                            nc.tensor.transpose(
                                pr[:],
                                ot[:, i * 128 : (i + 1) * 128],
                                ident[0 : H + 1, 0 : H + 1],
                            )
                            r = esb.tile([128, 1], f32, tag="r", bufs=2)
                            nc.vector.reciprocal(r[:], pr[:, H : H + 1])
                            nc.vector.tensor_scalar_mul(
                                x1rows[:, j, :], pr[:, 0:H], r[:]
                            )
                            pt = pst.tile([H, 128], f32, tag="pt")
                            nc.tensor.transpose(pt[:], x1rows[:, j, :], ident[:])
                            nc.vector.tensor_copy(
                                x_a[:, j * 128 : (j + 1) * 128], pt[:]
                            )
                nc.sync.dma_start(
                    d_sh[0].ap().rearrange("(c p) h -> p c h", p=128), x1rows[:]
                )

            # ---------------- SAGE layers (dense-A SpMM) ----------------
            for layer in range(2):
                x_in = x_a if layer == 0 else x_b
                x_out = x_b if layer == 0 else x_a

                nc.gpsimd.collective_compute(
                    "AllGather",
                    mybir.AluOpType.bypass,
                    replica_groups=[list(range(CORES))],
                    ins=[d_sh[layer][:]],
                    outs=[d_tb[layer][:]],
                )

                with (
                    tc.tile_pool(name=f"sage_sb{layer}", bufs=1) as ssb,
                    tc.tile_pool(name=f"sageA{layer}", bufs=3) as sA,
                    tc.tile_pool(name=f"ps_ag{layer}", bufs=1, space="PSUM") as pag,
                    tc.tile_pool(name=f"ps_t{layer}", bufs=1, space="PSUM") as pt2,
                    tc.tile_pool(name=f"ps_l{layer}", bufs=2, space="PSUM") as pl2,
                ):
                    # load the all-gathered table; build bf16 hi/lo split with
                    # ones (hi) / zeros (lo) in the augmented column
                    xtbl = ssb.tile([128, KC, H], f32)
                    nc.sync.dma_start(
                        xtbl[:], d_tb[layer].ap().rearrange("(k p) h -> p k h", p=128)
                    )
                    xhi = ssb.tile([128, KC, H + 1], bf16)
                    xlo = ssb.tile([128, KC, H + 1], bf16)
                    nc.vector.tensor_copy(xhi[:, :, 0:H], xtbl[:])
                    nc.vector.tensor_tensor(
                        xlo[:, :, 0:H],
                        xtbl[:],
                        xhi[:, :, 0:H],
                        op=mybir.AluOpType.subtract,
                    )
                    nc.vector.memset(xhi[:, :, H : H + 1], 1.0)
                    nc.vector.memset(xlo[:, :, H : H + 1], 0.0)

                    ag_ps = [
                        pag.tile([H + 1, 512], f32, tag=f"a{g}", name=f"ag_ps{layer}{g}")
                        for g in range(G)
                    ]
                    for k in range(KC):
                        Ak = sA.tile([128, NL], bf16, tag="Ak", name=f"Ak{layer}_{k}")
                        nc.sync.dma_start(Ak[:], t_A[k * 128 : (k + 1) * 128, :])
                        for g in range(G):
                            gs = slice(g * 512, (g + 1) * 512)
                            nc.tensor.matmul(
                                ag_ps[g][:],
                                xhi[:, k, :],
                                Ak[:, gs],
                                start=(k == 0),
                                stop=False,
                            )
                        for g in range(G):
                            gs = slice(g * 512, (g + 1) * 512)
                            nc.tensor.matmul(
                                ag_ps[g][:],
                                xlo[:, k, :],
                                Ak[:, gs],
                                start=False,
                                stop=(k == KC - 1),
                            )

                    # divide by deg (row H of each group), rebuild col layout
                    aggrT = ssb.tile([H, NL], f32)
                    for g in range(G):
                        ot2 = ssb.tile([H + 1, 512], f32, tag="ot2", bufs=2)
                        nc.scalar.copy(ot2[:], ag_ps[g][:])
                        for i in range(4):
                            j = g * 4 + i
                            pr2 = pt2.tile([128, H + 1], f32, tag="tp", bufs=2)
                            nc.tensor.transpose(
                                pr2[:],
                                ot2[:, i * 128 : (i + 1) * 128],
                                ident[0 : H + 1, 0 : H + 1],
                            )
                            dmax = ssb.tile([128, 1], f32, tag="dmax", bufs=2)
                            nc.vector.tensor_scalar_max(
                                dmax[:], pr2[:, H : H + 1], 1.0
                            )
                            drec = ssb.tile([128, 1], f32, tag="drec", bufs=2)
                            nc.vector.reciprocal(drec[:], dmax[:])
                            arows = ssb.tile([128, H], f32, tag="arows", bufs=2)
                            nc.vector.tensor_scalar_mul(
                                arows[:], pr2[:, 0:H], drec[:]
                            )
                            pt3 = pt2.tile([H, 128], f32, tag="tp", bufs=2, name=f"pt3_{layer}_{j}")
                            nc.tensor.transpose(pt3[:], arows[:], ident[:])
                            nc.vector.tensor_copy(
                                aggrT[:, j * 128 : (j + 1) * 128], pt3[:]
                            )

                    # sage linear + relu (col layout)
                    for g in range(G):
                        gs = slice(g * 512, (g + 1) * 512)
                        ps2 = pl2.tile([H, 512], f32, tag="sage")
                        nc.tensor.matmul(
                            ps2[:], wpack[:, WC_LW[layer]], aggrT[:, gs],
                            start=True, stop=False,
                        )
                        nc.tensor.matmul(
                            ps2[:], wpack[:, WC_RW[layer]], x_in[:, gs],
                            start=False, stop=True,
                        )
                        nc.scalar.activation(
                            x_out[:, gs],
                            ps2[:],
                            mybir.ActivationFunctionType.Relu,
                            bias=wpack[:, WC_LB[layer]],
                        )

                    # rows + store
                    xrows = ssb.tile([128, NB, H], f32, tag=f"xrows{layer}")
                    for j in range(NB):
                        pr3 = pt2.tile([128, H], f32, tag="tp", bufs=2, name=f"pr3_{layer}_{j}")
                        nc.tensor.transpose(
                            pr3[:],
                            x_out[:, j * 128 : (j + 1) * 128],
                            ident[0:H, 0:H],
                        )
                        nc.vector.tensor_copy(xrows[:, j, :], pr3[:])
                    dst = d_sh[1] if layer == 0 else t_out
                    nc.sync.dma_start(
                        dst.ap().rearrange("(c p) h -> p c h", p=128), xrows[:]
                    )

    nc.finalize()
    return nc


def _build_A(edge_index):
    """Per-core dense bf16 count matrices A[c][src, tgt_local]."""
    src = np.asarray(edge_index[0], dtype=np.int64)
    tgt = np.asarray(edge_index[1], dtype=np.int64)
    c = tgt // NL
    tloc = tgt % NL
    flat = (c * N + src) * NL + tloc
    uf, cnt = np.unique(flat, return_counts=True)
    A8 = np.zeros(CORES * N * NL, dtype=np.uint16)
    A8[uf] = cnt.astype(np.uint16)
    return A8.reshape(CORES, N, NL).astype(ml_dtypes.bfloat16)


def kernel(edge_index, user_emb, item_emb, attn_w, attn_b, causal_adj,
           l0_lw, l0_lb, l0_rw, l1_lw, l1_lb, l1_rw):
    global LAST_RESULTS
    edge_index = np.asarray(edge_index)
    user_emb = np.asarray(user_emb, dtype=np.float32)
    item_emb = np.asarray(item_emb, dtype=np.float32)
    attn_w = np.asarray(attn_w, dtype=np.float32)
    attn_b = np.asarray(attn_b, dtype=np.float32)
    causal_adj = np.asarray(causal_adj, dtype=np.float32)

    A = _build_A(edge_index)

    x0 = np.concatenate([user_emb, item_emb], axis=0)  # [N, H]
    x0aug = np.ascontiguousarray(
        np.concatenate([x0, np.ones((N, 1), np.float32)], axis=1)
    )
    # adj packed for PE row-group pairing: even key chunks on partitions
    # 0:64, odd chunks on 64:128
    a4 = causal_adj.reshape(H, KC // 2, 2, 128)
    adj2 = np.ascontiguousarray(
        np.concatenate([a4[:, :, 0, :], a4[:, :, 1, :]], axis=0).reshape(128, N // 2)
    )

    wpack = np.zeros((H, 323), dtype=np.float32)
    wpack[:, 0:64] = attn_w.T
    wpack[:, 64:128] = np.asarray(l0_lw, np.float32).T
    wpack[:, 128:192] = np.asarray(l0_rw, np.float32).T
    wpack[:, 192:256] = np.asarray(l1_lw, np.float32).T
    wpack[:, 256:320] = np.asarray(l1_rw, np.float32).T
    wpack[:, 320] = attn_b
    wpack[:, 321] = np.asarray(l0_lb, np.float32)
    wpack[:, 322] = np.asarray(l1_lb, np.float32)

    if "nc" not in _NC_CACHE:
        _NC_CACHE["nc"] = _build_nc()
    nc = _NC_CACHE["nc"]

    in_maps = []
    for c in range(CORES):
        xT0c = np.ascontiguousarray(x0[c * NL : (c + 1) * NL].T)
        in_maps.append(
            {
                "x0aug": x0aug,
                "xT0": xT0c,
                "adj2": adj2,
                "wpack": wpack,
                "A": A[c],
            }
        )

    res = run_bass_kernel_spmd(
        nc,
        in_maps,
        core_ids=list(range(CORES)),
        trace=bool(os.environ.get("KERNEL_TRACE")),
    )
    LAST_RESULTS = res

    out = np.concatenate([res.results[c]["out"] for c in range(CORES)], axis=0)
    return out[:U], out[U:]


# revision 27
# speedup vs baseline: 1.5407x; 1.0506x over previous
"""Trainium2 Bass kernel for CausalGNNRecommender.

Full inputs in, full outputs out. Internally: shard the N=16384 node dim
across 8 NeuronCores (2048 rows each) and run a fused
attention + 2x SAGE-conv kernel per core, with AllGather collectives
between layers.

Design notes:
- Attention is flash-style in column-major layout (H=64 on partitions):
  scoresT chunks [128 keys, 512 rows] on the PE (two K=64 matmuls packed
  into disjoint PE row-groups), exp on ACT batched [128, 1024], then
  PSUM-accumulated attn @ x with a ones-column giving softmax
  normalizers for free (no max subtraction needed: |scores| <= ~35 so
  exp stays in fp32 range).
- SAGE segment-sum is a dense SpMM: the host re-encodes edge_index as a
  per-core bf16 count matrix A [N, 2048] (counts are exact in bf16);
  the device streams A as the matmul moving operand against the node
  features split as x = hi + lo bf16 stationaries (recovers full fp32
  precision). A ones-column in x_hi yields in-degrees for free.
"""

import os
import sys

sys.path.insert(0, "/opt/trn_rl_repo")

import numpy as np
import ml_dtypes

import concourse.bacc as bacc
import concourse.tile as tile
from concourse import mybir
from concourse.bass_utils import run_bass_kernel_spmd
from concourse.masks import make_identity

# Problem shapes (hardcoded per harness contract).
U, I, H, E = 4096, 12288, 64, 524288
N = U + I              # 16384
CORES = 8
NL = N // CORES        # 2048 rows per core
NB = NL // 128         # 16 node blocks per core
KC = N // 128          # 128 key chunks
G = NL // 512          # 4 groups of 512 rows
CH = 4                 # load/split chunking along the key dim
KCC = KC // CH         # key chunks per load chunk (32)

f32 = mybir.dt.float32
bf16 = mybir.dt.bfloat16

# Results of the last device run (exec time etc.) for external inspection.
LAST_RESULTS = None
_NC_CACHE = {}

# wpack columns
WC_ATTN = slice(0, 64)
WC_LW = [slice(64, 128), slice(192, 256)]
WC_RW = [slice(128, 192), slice(256, 320)]
WC_AB = slice(320, 321)
WC_LB = [slice(321, 322), slice(322, 323)]


def _build_nc():
    """Build the single-core SPMD Bass program (identical on all 8 cores)."""
    nc = bacc.Bacc("TRN2", target_bir_lowering=False, debug=False)

    t_x0aug = nc.dram_tensor("x0aug", [N, H + 1], f32, kind="ExternalInput")
    t_xT0 = nc.dram_tensor("xT0", [H, NL], f32, kind="ExternalInput")
    t_adj2 = nc.dram_tensor("adj2", [128, N // 2], f32, kind="ExternalInput")
    t_wpack = nc.dram_tensor("wpack", [H, 323], f32, kind="ExternalInput")
    t_A = nc.dram_tensor("A", [N, NL], bf16, kind="ExternalInput")
    t_out = nc.dram_tensor("out", [NL, H], f32, kind="ExternalOutput")

    d_sh = [nc.dram_tensor(f"sh{l}", [NL, H], f32) for l in range(2)]
    d_tb = [
        nc.dram_tensor(f"tb{l}", [N, H], f32, addr_space="Shared")
        for l in range(2)
    ]

    with tile.TileContext(nc) as tc:
        with tc.tile_pool(name="consts", bufs=1) as consts:
            ident = consts.tile([128, 128], f32)
            make_identity(nc, ident[:])
            wpack = consts.tile([H, 323], f32)
            nc.sync.dma_start(wpack[:], t_wpack[:])
            xT0 = consts.tile([H, NL], f32)
            nc.sync.dma_start(xT0[:], t_xT0[:])
            x_a = consts.tile([H, NL], f32)  # x1T (attention out, col layout)
            x_b = consts.tile([H, NL], f32)  # x2T (layer-0 out, col layout)

            # ---------------- attention ----------------
            with (
                tc.tile_pool(name="attn_sb", bufs=1) as asb,
                tc.tile_pool(name="exp_sb", bufs=6) as esb,
                tc.tile_pool(name="ps_out", bufs=1, space="PSUM") as pso,
            ):
                f32r = mybir.dt.float32r
                x0aug = [
                    asb.tile([128, KCC, H + 1], f32r, name=f"x0aug{c}")
                    for c in range(CH)
                ]
                adj2 = [
                    asb.tile([128, KCC * 128 // 2], f32r, name=f"adj2{c}")
                    for c in range(CH)
                ]
                with tc.tile_pool(name="raw_sb", bufs=2) as rsb:
                    for c in range(CH):
                        rawa = rsb.tile(
                            [128, KCC * 64], f32, tag="ra", name=f"rawa{c}"
                        )
                        nc.sync.dma_start(
                            rawa[:], t_adj2[:, c * KCC * 64 : (c + 1) * KCC * 64]
                        )
                        nc.vector.tensor_copy(adj2[c][:], rawa[:])
                        rawx = rsb.tile(
                            [128, KCC, H + 1], f32, tag="rx", name=f"rawx{c}"
                        )
                        nc.sync.dma_start(
                            rawx[:],
                            t_x0aug.ap().rearrange("(k p) h -> p k h", p=128)[
                                :, c * KCC : (c + 1) * KCC, :
                            ],
                        )
                        nc.vector.tensor_copy(x0aug[c][:], rawx[:])

                # xWT = attn_w @ xT0 + b, duplicated onto partitions 64:128
                xWT = asb.tile([128, NL], f32r)
                with tc.tile_pool(name="ps_w", bufs=2, space="PSUM") as psw:
                    for g in range(G):
                        gs = slice(g * 512, (g + 1) * 512)
                        ps = psw.tile([128, 512], f32, tag="xw")
                        nc.tensor.matmul(ps[0:64, :], wpack[:, WC_ATTN], xT0[:, gs])
                        nc.tensor.matmul(ps[64:128, :], wpack[:, WC_ATTN], xT0[:, gs])
                        nc.scalar.activation(
                            xWT[0:64, gs],
                            ps[0:64, :],
                            mybir.ActivationFunctionType.Identity,
                            bias=wpack[:, WC_AB],
                        )
                        nc.scalar.activation(
                            xWT[64:128, gs],
                            ps[64:128, :],
                            mybir.ActivationFunctionType.Identity,
                            bias=wpack[:, WC_AB],
                        )

                out_ps = [
                    pso.tile([H + 1, 512], f32, tag=f"o{g}", name=f"out_ps{g}")
                    for g in range(G)
                ]

                # main loop: k2 indexes pairs of key chunks (2*k2, 2*k2+1)
                with tc.tile_pool(name="ps_sc", bufs=2, space="PSUM") as pss:
                    for k2 in range(KC // 2):
                        c = (2 * k2) // KCC
                        adjc = adj2[c]
                        xac = x0aug[c]
                        kof = k2 - c * (KCC // 2)
                        ks = slice(kof * 128, (kof + 1) * 128)
                        exs = []
                        for g in range(G):
                            gs = slice(g * 512, (g + 1) * 512)
                            sc = pss.tile([128, 2, 512], f32, tag="sc")
                            # two K=64 matmuls packed into PE row groups 0/64
                            nc.tensor.matmul(
                                sc[:, 0, :], adjc[0:64, ks], xWT[0:64, gs]
                            )
                            nc.tensor.matmul(
                                sc[:, 1, :], adjc[64:128, ks], xWT[64:128, gs]
                            )
                            ex = esb.tile([128, 2, 512], f32r, tag="ex", bufs=6)
                            nc.scalar.activation(
                                ex[:], sc[:], mybir.ActivationFunctionType.Exp
                            )
                            exs.append(ex)
                        for g in range(G):
                            nc.tensor.matmul(
                                out_ps[g][:],
                                xac[:, 2 * k2 - c * KCC, :],
                                exs[g][:, 0, :],
                                start=(k2 == 0),
                                stop=False,
                            )
                            nc.tensor.matmul(
                                out_ps[g][:],
                                xac[:, 2 * k2 + 1 - c * KCC, :],
                                exs[g][:, 1, :],
                                start=False,
                                stop=(k2 == KC // 2 - 1),
                            )

                # softmax divide + build x1 rows and x1T
                x1rows = asb.tile([128, NB, H], f32)
                with tc.tile_pool(name="ps_tr", bufs=2, space="PSUM") as pst:
                    for g in range(G):
                        ot = esb.tile([H + 1, 512], f32, tag="ot", bufs=2)
                        nc.scalar.copy(ot[:], out_ps[g][:])
                        for i in range(4):
                            j = g * 4 + i
                            pr = pst.tile([128, H + 1], f32, tag="pr")
                            nc.tensor.transpose(
                                pr[:],
                                ot[:, i * 128 : (i + 1) * 128],
                                ident[0 : H + 1, 0 : H + 1],
                            )
                            r = esb.tile([128, 1], f32, tag="r", bufs=2)
                            nc.vector.reciprocal(r[:], pr[:, H : H + 1])
                            nc.vector.tensor_scalar_mul(
                                x1rows[:, j, :], pr[:, 0:H], r[:]
                            )
                            pt = pst.tile([H, 128], f32, tag="pt")
                            nc.tensor.transpose(pt[:], x1rows[:, j, :], ident[:])
                            nc.vector.tensor_copy(
                                x_a[:, j * 128 : (j + 1) * 128], pt[:]
                            )
                nc.sync.dma_start(
                    d_sh[0].ap().rearrange("(c p) h -> p c h", p=128), x1rows[:]
                )

            # ---------------- SAGE layers (dense-A SpMM) ----------------
            for layer in range(2):
                x_in = x_a if layer == 0 else x_b
                x_out = x_b if layer == 0 else x_a

                nc.gpsimd.collective_compute(
                    "AllGather",
                    mybir.AluOpType.bypass,
                    replica_groups=[list(range(CORES))],
                    ins=[d_sh[layer][:]],
                    outs=[d_tb[layer][:]],
                )

                with (
                    tc.tile_pool(name=f"sage_sb{layer}", bufs=1) as ssb,
                    tc.tile_pool(name=f"sageA{layer}", bufs=3) as sA,
                    tc.tile_pool(name=f"ps_ag{layer}", bufs=1, space="PSUM") as pag,
                    tc.tile_pool(name=f"ps_t{layer}", bufs=1, space="PSUM") as pt2,
                    tc.tile_pool(name=f"ps_l{layer}", bufs=2, space="PSUM") as pl2,
                ):
                    # load the all-gathered table chunked; build bf16 hi/lo
                    # split with ones (hi) / zeros (lo) in the aug column
                    xhi = [
                        ssb.tile([128, KCC, H + 1], bf16, name=f"xhi{layer}_{c}")
                        for c in range(CH)
                    ]
                    xlo = [
                        ssb.tile([128, KCC, H + 1], bf16, name=f"xlo{layer}_{c}")
                        for c in range(CH)
                    ]
                    for c in range(CH):
                        xt = ssb.tile(
                            [128, KCC, H], f32, tag="xt", bufs=2,
                            name=f"xt{layer}_{c}",
                        )
                        nc.sync.dma_start(
                            xt[:],
                            d_tb[layer].ap().rearrange("(k p) h -> p k h", p=128)[
                                :, c * KCC : (c + 1) * KCC, :
                            ],
                        )
                        nc.vector.tensor_copy(xhi[c][:, :, 0:H], xt[:])
                        nc.vector.tensor_tensor(
                            xlo[c][:, :, 0:H],
                            xt[:],
                            xhi[c][:, :, 0:H],
                            op=mybir.AluOpType.subtract,
                        )
                        nc.vector.memset(xhi[c][:, :, H : H + 1], 1.0)
                        nc.vector.memset(xlo[c][:, :, H : H + 1], 0.0)

                    ag_ps = [
                        pag.tile(
                            [H + 1, 512], f32, tag=f"a{g}", name=f"ag_ps{layer}{g}"
                        )
                        for g in range(G)
                    ]
                    for k in range(KC):
                        c = k // KCC
                        Ak = sA.tile([128, NL], bf16, tag="Ak", name=f"Ak{layer}_{k}")
                        nc.sync.dma_start(Ak[:], t_A[k * 128 : (k + 1) * 128, :])
                        for g in range(G):
                            nc.tensor.matmul(
                                ag_ps[g][:],
                                xhi[c][:, k - c * KCC, :],
                                Ak[:, g * 512 : (g + 1) * 512],
                                start=(k == 0),
                                stop=False,
                            )
                        for g in range(G):
                            nc.tensor.matmul(
                                ag_ps[g][:],
                                xlo[c][:, k - c * KCC, :],
                                Ak[:, g * 512 : (g + 1) * 512],
                                start=False,
                                stop=(k == KC - 1),
                            )

                    # divide by deg (row H), rebuild col layout per group
                    aggrT = [
                        ssb.tile([H, 512], f32, name=f"aggrT{layer}_{g}")
                        for g in range(G)
                    ]
                    for g in range(G):
                        ot2 = ssb.tile(
                            [H + 1, 512], f32, tag="ot2", bufs=2,
                            name=f"ot2_{layer}{g}",
                        )
                        nc.scalar.copy(ot2[:], ag_ps[g][:])
                        for i in range(4):
                            j = g * 4 + i
                            pr2 = pt2.tile([128, H + 1], f32, tag="tp", bufs=2)
                            nc.tensor.transpose(
                                pr2[:],
                                ot2[:, i * 128 : (i + 1) * 128],
                                ident[0 : H + 1, 0 : H + 1],
                            )
                            dmax = ssb.tile([128, 1], f32, tag="dmax", bufs=2)
                            nc.vector.tensor_scalar_max(
                                dmax[:], pr2[:, H : H + 1], 1.0
                            )
                            drec = ssb.tile([128, 1], f32, tag="drec", bufs=2)
                            nc.vector.reciprocal(drec[:], dmax[:])
                            arows = ssb.tile([128, H], f32, tag="arows", bufs=2)
                            nc.vector.tensor_scalar_mul(
                                arows[:], pr2[:, 0:H], drec[:]
                            )
                            pt3 = pt2.tile([H, 128], f32, tag="tp", bufs=2, name=f"pt3_{layer}_{j}")
                            nc.tensor.transpose(pt3[:], arows[:], ident[:])
                            nc.vector.tensor_copy(
                                aggrT[g][:, (j % 4) * 128 : (j % 4 + 1) * 128],
                                pt3[:],
                            )

                    # sage linear + relu (col layout)
                    for g in range(G):
                        gs = slice(g * 512, (g + 1) * 512)
                        ps2 = pl2.tile([H, 512], f32, tag="sage")
                        nc.tensor.matmul(
                            ps2[:], wpack[:, WC_LW[layer]], aggrT[g][:],
                            start=True, stop=False,
                        )
                        nc.tensor.matmul(
                            ps2[:], wpack[:, WC_RW[layer]], x_in[:, gs],
                            start=False, stop=True,
                        )
                        nc.scalar.activation(
                            x_out[:, gs],
                            ps2[:],
                            mybir.ActivationFunctionType.Relu,
                            bias=wpack[:, WC_LB[layer]],
                        )

                    # rows + store
                    xrows = ssb.tile([128, NB, H], f32, tag=f"xrows{layer}")
                    for j in range(NB):
                        pr3 = pt2.tile([128, H], f32, tag="tp", bufs=2, name=f"pr3_{layer}_{j}")
                        nc.tensor.transpose(
                            pr3[:],
                            x_out[:, j * 128 : (j + 1) * 128],
                            ident[0:H, 0:H],
                        )
                        nc.vector.tensor_copy(xrows[:, j, :], pr3[:])
                    dst = d_sh[1] if layer == 0 else t_out
                    nc.sync.dma_start(
                        dst.ap().rearrange("(c p) h -> p c h", p=128), xrows[:]
                    )

    nc.finalize()
    return nc


def _build_A(edge_index):
    """Per-core dense bf16 count matrices A[c][src, tgt_local]."""
    src = np.asarray(edge_index[0], dtype=np.int64)
    tgt = np.asarray(edge_index[1], dtype=np.int64)
    c = tgt // NL
    tloc = tgt % NL
    flat = (c * N + src) * NL + tloc
    uf, cnt = np.unique(flat, return_counts=True)
    A8 = np.zeros(CORES * N * NL, dtype=np.uint16)
    A8[uf] = cnt.astype(np.uint16)
    return A8.reshape(CORES, N, NL).astype(ml_dtypes.bfloat16)


def kernel(edge_index, user_emb, item_emb, attn_w, attn_b, causal_adj,
           l0_lw, l0_lb, l0_rw, l1_lw, l1_lb, l1_rw):
    global LAST_RESULTS
    edge_index = np.asarray(edge_index)
    user_emb = np.asarray(user_emb, dtype=np.float32)
    item_emb = np.asarray(item_emb, dtype=np.float32)
    attn_w = np.asarray(attn_w, dtype=np.float32)
    attn_b = np.asarray(attn_b, dtype=np.float32)
    causal_adj = np.asarray(causal_adj, dtype=np.float32)

    A = _build_A(edge_index)

    x0 = np.concatenate([user_emb, item_emb], axis=0)  # [N, H]
    x0aug = np.ascontiguousarray(
        np.concatenate([x0, np.ones((N, 1), np.float32)], axis=1)
    )
    # adj packed for PE row-group pairing: even key chunks on partitions
    # 0:64, odd chunks on 64:128
    a4 = causal_adj.reshape(H, KC // 2, 2, 128)
    adj2 = np.ascontiguousarray(
        np.concatenate([a4[:, :, 0, :], a4[:, :, 1, :]], axis=0).reshape(128, N // 2)
    )

    wpack = np.zeros((H, 323), dtype=np.float32)
    wpack[:, 0:64] = attn_w.T
    wpack[:, 64:128] = np.asarray(l0_lw, np.float32).T
    wpack[:, 128:192] = np.asarray(l0_rw, np.float32).T
    wpack[:, 192:256] = np.asarray(l1_lw, np.float32).T
    wpack[:, 256:320] = np.asarray(l1_rw, np.float32).T
    wpack[:, 320] = attn_b
    wpack[:, 321] = np.asarray(l0_lb, np.float32)
    wpack[:, 322] = np.asarray(l1_lb, np.float32)

    if "nc" not in _NC_CACHE:
        _NC_CACHE["nc"] = _build_nc()
    nc = _NC_CACHE["nc"]

    in_maps = []
    for c in range(CORES):
        xT0c = np.ascontiguousarray(x0[c * NL : (c + 1) * NL].T)
        in_maps.append(
            {
                "x0aug": x0aug,
                "xT0": xT0c,
                "adj2": adj2,
                "wpack": wpack,
                "A": A[c],
            }
        )

    res = run_bass_kernel_spmd(
        nc,
        in_maps,
        core_ids=list(range(CORES)),
        trace=bool(os.environ.get("KERNEL_TRACE")),
    )
    LAST_RESULTS = res

    out = np.concatenate([res.results[c]["out"] for c in range(CORES)], axis=0)
    return out[:U], out[U:]
